# revision 11
# baseline (speedup 1.0000x reference)
"""DTFD-MIL Trainium2 kernel.

Sharding: one group (8192 instances) per NeuronCore, 8 cores.
Stage 1 (SPMD, 8 cores): per group g —
    midT = relu(w_dim^T-contraction of x_g)  computed in [f, m] layout (fp32r),
    gated attention scores a, t = mid@dw, c2 = mid@sub_w (all PE),
    e = exp(a) (unnormalized softmax weights; ranking and the final
    normalized sums never need the max-subtraction),
    s = e*t ranking scores, top-8/bottom-8 via vector.max/max_index,
    indirect-gather the 16 selected x rows, recompute their mid rows
    (pseudo features), pred_g = sum(e*c2)/Z + sub_b.
Stage 2 (1 core): gated attention over the 128 pseudo rows -> afeat, bag_pred.
"""
import sys

sys.path.insert(0, "/opt/trn_rl_repo")

import numpy as np
import concourse.bass as bass
import concourse.mybir as mybir
import concourse.tile as tile
from concourse import bacc
from concourse.bass import ts
from concourse.bass_utils import run_bass_kernel_spmd

F32 = mybir.dt.float32
F32R = mybir.dt.float32r
U32 = mybir.dt.uint32
AF = mybir.ActivationFunctionType

N, F, D, C, G = 65536, 1024, 128, 2, 8
NG = N // G            # 8192 instances per group/core
MC = 256               # m-chunk
NCHUNK = NG // MC      # 32
KB = F // 128          # 8 k-blocks


def build_stage1():
    nc = bacc.Bacc("TRN2", target_bir_lowering=False, debug=False)
    xg_d = nc.dram_tensor("xg", [NG, F], F32, kind="ExternalInput")
    w_d = nc.dram_tensor("w_dim", [F, F], F32, kind="ExternalInput")
    attw_d = nc.dram_tensor("attw", [F, 259], F32, kind="ExternalInput")
    attw3_d = nc.dram_tensor("attw3", [D, 3], F32, kind="ExternalInput")
    wb3_d = nc.dram_tensor("wb3", [3, 1], F32, kind="ExternalInput")
    vb_d = nc.dram_tensor("vb", [D, 1], F32, kind="ExternalInput")
    ub_d = nc.dram_tensor("ub", [D, 1], F32, kind="ExternalInput")
    subb3_d = nc.dram_tensor("subb3", [3, 1], F32, kind="ExternalInput")
    id_d = nc.dram_tensor("ident", [128, 128], F32, kind="ExternalInput")

    pred_d = nc.dram_tensor("pred", [3, 1], F32, kind="ExternalOutput")
    pseudo_d = nc.dram_tensor("pseudo", [16, F], F32, kind="ExternalOutput")
    idx_d = nc.dram_tensor("idx", [1, 16], U32, kind="ExternalOutput")

    with tile.TileContext(nc) as tc:
        with (
            tc.tile_pool(name="const", bufs=1) as cpool,
            tc.tile_pool(name="stage", bufs=1) as stpool,
            tc.tile_pool(name="xrow", bufs=4) as xrow_pool,
            tc.tile_pool(name="xt", bufs=2) as xt_pool,
            tc.tile_pool(name="midt", bufs=2) as midt_pool,
            tc.tile_pool(name="att", bufs=2) as att_pool,
            tc.tile_pool(name="acc", bufs=1) as acc_pool,
            tc.tile_pool(name="tail", bufs=1) as tail_pool,
            tc.tile_pool(name="dram", bufs=1, space="DRAM") as dram_pool,
            tc.tile_pool(name="ps_xt", bufs=2, space="PSUM") as ps_xt,
            tc.tile_pool(name="ps_mid", bufs=2, space="PSUM") as ps_mid,
            tc.tile_pool(name="ps_vu", bufs=2, space="PSUM") as ps_vu,
            tc.tile_pool(name="ps_misc", bufs=2, space="PSUM") as ps_misc,
        ):
            # ---- constants / weights ----
            ident = cpool.tile([128, 128], F32)
            nc.sync.dma_start(ident[:], id_d.ap())

            w_r = cpool.tile([128, KB, F], F32R)       # w_dim, [k-part, kb, n]
            stage = stpool.tile([128, KB * F], F32, tag="stage32")
            nc.sync.dma_start(
                stage[:].rearrange("p (kb n) -> p kb n", kb=KB),
                w_d.ap().rearrange("(kb p) n -> p kb n", p=128),
            )
            nc.vector.tensor_copy(
                w_r[:].rearrange("p kb n -> p (kb n)"), stage[:]
            )

            attw_r = cpool.tile([128, KB, 259], F32R)  # [Vw | Uw | dw sw0 sw1]
            stage2 = stpool.tile([128, KB * 259], F32, tag="stage32")
            nc.sync.dma_start(
                stage2[:].rearrange("p (kb n) -> p kb n", kb=KB),
                attw_d.ap().rearrange("(kb p) n -> p kb n", p=128),
            )
            nc.vector.tensor_copy(
                attw_r[:].rearrange("p kb n -> p (kb n)"), stage2[:]
            )

            attw3_r = cpool.tile([D, 3], F32R)
            st3 = cpool.tile([D, 3], F32)
            nc.sync.dma_start(st3[:], attw3_d.ap())
            nc.vector.tensor_copy(attw3_r[:], st3[:])

            wb3 = cpool.tile([3, 1], F32)
            nc.sync.dma_start(wb3[:], wb3_d.ap())
            vb = cpool.tile([D, 1], F32)
            nc.sync.dma_start(vb[:], vb_d.ap())
            ub = cpool.tile([D, 1], F32)
            nc.sync.dma_start(ub[:], ub_d.ap())
            subb3 = cpool.tile([3, 1], F32)
            nc.sync.dma_start(subb3[:], subb3_d.ap())

            # ---- persistent accumulators ----
            s_all = acc_pool.tile([1, NG], F32)
            zpart = acc_pool.tile([3, NCHUNK], F32)
            pc2z = acc_pool.tile([3, NCHUNK], F32)

            # ---- main loop over m-chunks ----
            for j in range(NCHUNK):
                xrows = [xrow_pool.tile([128, F], F32, tag="xrow", name=f"xr{j}_{h}") for h in range(2)]
                for h in range(2):
                    nc.sync.dma_start(
                        xrows[h][:], xg_d.ap()[j * MC + h * 128 : j * MC + (h + 1) * 128, :]
                    )
                # transpose x rows -> xT [k-part, kb, m]
                xt = xt_pool.tile([128, KB, MC], F32R)
                for h in range(2):
                    for kb in range(KB):
                        pxt = ps_xt.tile([128, 128], F32)
                        nc.tensor.transpose(pxt[:], xrows[h][:, ts(kb, 128)], ident[:])
                        eng = nc.vector if (kb + h) % 2 == 0 else nc.scalar
                        if eng is nc.vector:
                            nc.vector.tensor_copy(
                                xt[:, kb, h * 128 : (h + 1) * 128], pxt[:]
                            )
                        else:
                            nc.scalar.copy(xt[:, kb, h * 128 : (h + 1) * 128], pxt[:])

                # GEMM1: midT[nt] = relu(sum_kb w[kb,nt]^T @ xT[kb])
                midt = midt_pool.tile([128, KB, MC], F32R)
                for nt in range(KB):
                    pm = ps_mid.tile([128, MC], F32)
                    for kb in range(KB):
                        nc.tensor.matmul(
                            pm[:],
                            w_r[:, kb, ts(nt, 128)],
                            xt[:, kb, :],
                            start=(kb == 0),
                            stop=(kb == KB - 1),
                        )
                    nc.scalar.activation(midt[:, nt, :], pm[:], AF.Relu)

                # attention GEMMs over f: VT/UT [d, m], tc2 [3, m]
                pv = ps_vu.tile([128, MC], F32, tag="vu")
                pu = ps_vu.tile([128, MC], F32, tag="vu")
                ptc = ps_misc.tile([3, MC], F32, tag="misc")
                for kb in range(KB):
                    nc.tensor.matmul(
                        pv[:], attw_r[:, kb, 0:128], midt[:, kb, :],
                        start=(kb == 0), stop=(kb == KB - 1),
                    )
                for kb in range(KB):
                    nc.tensor.matmul(
                        pu[:], attw_r[:, kb, 128:256], midt[:, kb, :],
                        start=(kb == 0), stop=(kb == KB - 1),
                    )
                for kb in range(KB):
                    nc.tensor.matmul(
                        ptc[:], attw_r[:, kb, 256:259], midt[:, kb, :],
                        start=(kb == 0), stop=(kb == KB - 1),
                    )
                vt = att_pool.tile([128, MC], F32, tag="vt")
                nc.scalar.activation(vt[:], pv[:], AF.Tanh, bias=vb[:])
                st = att_pool.tile([128, MC], F32, tag="st")
                nc.scalar.activation(st[:], pu[:], AF.Sigmoid, bias=ub[:])
                g = att_pool.tile([128, MC], F32R, tag="g")
                nc.vector.tensor_mul(g[:], vt[:], st[:])

                pa = ps_misc.tile([3, MC], F32, tag="misc")
                nc.tensor.matmul(pa[:], attw3_r[:], g[:], start=True, stop=True)
                e3 = att_pool.tile([3, MC], F32, tag="e3")
                nc.scalar.activation(
                    e3[:], pa[:], AF.Exp, bias=wb3[:], accum_out=zpart[:, j : j + 1]
                )
                prod = att_pool.tile([3, MC], F32, tag="prod")
                nc.vector.tensor_mul(prod[:], e3[:], ptc[:])
                nc.vector.tensor_copy(s_all[:, j * MC : (j + 1) * MC], prod[0:1, :])
                nc.vector.reduce_sum(
                    pc2z[:, j : j + 1], prod[:], axis=mybir.AxisListType.X
                )

            # ---- tail: topk, gather, recompute pseudo, pred ----
            idx16 = tail_pool.tile([1, 16], U32)
            tv = tail_pool.tile([1, 8], F32)
            nc.vector.max(out=tv[:], in_=s_all[:])
            nc.vector.max_index(out=idx16[:, 0:8], in_max=tv[:], in_values=s_all[:])
            sneg = stpool.tile([1, NG], F32, tag="stage32")
            nc.vector.tensor_scalar_mul(sneg[:], s_all[:], -1.0)
            bv = tail_pool.tile([1, 8], F32)
            nc.vector.max(out=bv[:], in_=sneg[:])
            nc.vector.max_index(out=idx16[:, 8:16], in_max=bv[:], in_values=sneg[:])
            nc.sync.dma_start(idx_d.ap(), idx16[:])

            dscr = dram_pool.tile([16, 1], U32)
            nc.sync.dma_start(dscr[:].rearrange("a b -> b a"), idx16[:])
            idxp = tail_pool.tile([16, 1], U32)
            nc.sync.dma_start(idxp[:], dscr[:])

            xsel = tail_pool.tile([16, F], F32)
            nc.gpsimd.indirect_dma_start(
                out=xsel[:],
                out_offset=None,
                in_=xg_d.ap(),
                in_offset=bass.IndirectOffsetOnAxis(ap=idxp[:], axis=0),
            )
            xselt = tail_pool.tile([128, KB, 16], F32R)
            for kb in range(KB):
                pxt = ps_xt.tile([128, 128], F32)
                nc.tensor.transpose(
                    pxt[:, 0:16], xsel[:, ts(kb, 128)], ident[0:16, 0:16]
                )
                nc.vector.tensor_copy(xselt[:, kb, :], pxt[:, 0:16])
            pseudo = tail_pool.tile([16, F], F32)
            for nb in range(2):
                pp = ps_mid.tile([16, 512], F32, tag="pm")
                for kb in range(KB):
                    nc.tensor.matmul(
                        pp[:],
                        xselt[:, kb, :],
                        w_r[:, kb, nb * 512 : (nb + 1) * 512],
                        start=(kb == 0),
                        stop=(kb == KB - 1),
                    )
                nc.scalar.activation(pseudo[:, nb * 512 : (nb + 1) * 512], pp[:], AF.Relu)
            nc.sync.dma_start(pseudo_d.ap(), pseudo[:])

            zsum = tail_pool.tile([3, 1], F32)
            nc.vector.reduce_sum(zsum[:], zpart[:], axis=mybir.AxisListType.X)
            rz = tail_pool.tile([1, 1], F32)
            nc.vector.reciprocal(rz[:], zsum[0:1, :])
            rz3 = tail_pool.tile([3, 1], F32)
            nc.gpsimd.partition_broadcast(rz3[:], rz[:])
            pcs = tail_pool.tile([3, 1], F32)
            nc.vector.reduce_sum(pcs[:], pc2z[:], axis=mybir.AxisListType.X)
            pred = tail_pool.tile([3, 1], F32)
            nc.vector.tensor_mul(pred[:], pcs[:], rz3[:])
            nc.vector.tensor_add(pred[:], pred[:], subb3[:])
            nc.sync.dma_start(pred_d.ap(), pred[:])
    nc.compile()
    return nc


def build_stage2():
    nc = bacc.Bacc("TRN2", target_bir_lowering=False, debug=False)
    ps_d = nc.dram_tensor("pseudo_all", [128, F], F32, kind="ExternalInput")
    a2w_d = nc.dram_tensor("a2w", [F, 258], F32, kind="ExternalInput")
    a2wrow_d = nc.dram_tensor("a2wrow", [1, D], F32, kind="ExternalInput")
    b2row_d = nc.dram_tensor("b2row", [1, 256], F32, kind="ExternalInput")
    b2s_d = nc.dram_tensor("b2s", [128, 1], F32, kind="ExternalInput")
    clsb_d = nc.dram_tensor("cls_b", [1, C], F32, kind="ExternalInput")
    zb1_d = nc.dram_tensor("zb_ones", [128, 16], F32, kind="ExternalInput")
    id_d = nc.dram_tensor("ident", [128, 128], F32, kind="ExternalInput")

    afeat_d = nc.dram_tensor("afeat", [1, F], F32, kind="ExternalOutput")
    bag_d = nc.dram_tensor("bag", [1, C], F32, kind="ExternalOutput")

    with tile.TileContext(nc) as tc:
        with (
            tc.tile_pool(name="sb", bufs=1) as sb,
            tc.tile_pool(name="ps", bufs=1, space="PSUM") as ps,
            tc.tile_pool(name="ps2", bufs=2, space="PSUM") as ps2,
        ):
            ident = sb.tile([128, 128], F32)
            nc.sync.dma_start(ident[:], id_d.ap())
            psf = sb.tile([128, F], F32)
            nc.sync.dma_start(psf[:], ps_d.ap())
            ps_r = sb.tile([128, F], F32R)
            nc.vector.tensor_copy(ps_r[:], psf[:])

            a2w = sb.tile([128, KB, 258], F32)
            nc.sync.dma_start(
                a2w[:], a2w_d.ap().rearrange("(kb p) n -> p kb n", p=128)
            )
            a2w_r = sb.tile([128, KB, 258], F32R)
            nc.vector.tensor_copy(
                a2w_r[:].rearrange("p kb n -> p (kb n)"),
                a2w[:].rearrange("p kb n -> p (kb n)"),
            )
            a2wrow = sb.tile([1, D], F32)
            nc.sync.dma_start(a2wrow[:], a2wrow_d.ap())
            b2row = sb.tile([1, 256], F32)
            nc.sync.dma_start(b2row[:], b2row_d.ap())
            b2s = sb.tile([128, 1], F32)
            nc.sync.dma_start(b2s[:], b2s_d.ap())
            clsb = sb.tile([1, C], F32)
            nc.sync.dma_start(clsb[:], clsb_d.ap())
            zb1 = sb.tile([128, 16], F32)
            nc.sync.dma_start(zb1[:], zb1_d.ap())

            # pseudoT
            pst = sb.tile([128, KB, 128], F32R)
            for kb in range(KB):
                pt = ps2.tile([128, 128], F32, tag="t")
                nc.tensor.transpose(pt[:], psf[:, ts(kb, 128)], ident[:])
                nc.vector.tensor_copy(pst[:, kb, :], pt[:])

            # [V2 | U2 | pseudo@cls_w] = pseudoT-stationary x weights-moving
            pvu = ps.tile([128, 258], F32, tag="vu")
            for kb in range(KB):
                nc.tensor.matmul(
                    pvu[:], pst[:, kb, :], a2w_r[:, kb, :],
                    start=(kb == 0), stop=(kb == KB - 1),
                )
            b2b = sb.tile([128, 256], F32)
            nc.gpsimd.partition_broadcast(b2b[:], b2row[:])
            vu = sb.tile([128, 256], F32)
            nc.vector.tensor_add(vu[:], pvu[:, 0:256], b2b[:])
            th = sb.tile([128, 128], F32)
            nc.scalar.activation(th[:], vu[:, 0:128], AF.Tanh)
            sg = sb.tile([128, 128], F32)
            nc.scalar.activation(sg[:], vu[:, 128:256], AF.Sigmoid)
            g2 = sb.tile([128, 128], F32)
            nc.vector.tensor_mul(g2[:], th[:], sg[:])
            g2w = sb.tile([128, 128], F32)
            a2wb = sb.tile([128, 128], F32)
            nc.gpsimd.partition_broadcast(a2wb[:], a2wrow[:])
            nc.vector.tensor_mul(g2w[:], g2[:], a2wb[:])
            a2 = sb.tile([128, 1], F32)
            nc.vector.reduce_sum(a2[:], g2w[:], axis=mybir.AxisListType.X)
            e2 = sb.tile([128, 1], F32R)
            nc.scalar.activation(e2[:], a2[:], AF.Exp, bias=b2s[:])
            e2_2 = sb.tile([128, 2], F32R)
            nc.vector.tensor_copy(e2_2[:, 0:1], e2[:])
            nc.vector.tensor_copy(e2_2[:, 1:2], e2[:])

            # zb = [ones(14) | bcT(2)]; one matmul row0 -> [Z2 x14, bag_un x2]
            zb_r = sb.tile([128, 16], F32R)
            nc.vector.tensor_copy(zb_r[:, 0:14], zb1[:, 0:14])
            nc.vector.tensor_copy(zb_r[:, 14:16], pvu[:, 256:258])
            pzb = ps2.tile([2, 16], F32, tag="zb")
            nc.tensor.matmul(pzb[:], e2_2[:], zb_r[:], start=True, stop=True)
            rz2 = sb.tile([1, 1], F32)
            nc.vector.reciprocal(rz2[:], pzb[0:1, 0:1])

            afeat = sb.tile([1, F], F32)
            for nb in range(2):
                paf = ps2.tile([2, 512], F32, tag="af")
                nc.tensor.matmul(
                    paf[:], e2_2[:], ps_r[:, nb * 512 : (nb + 1) * 512],
                    start=True, stop=True,
                )
                nc.scalar.activation(
                    afeat[:, nb * 512 : (nb + 1) * 512], paf[0:1, :], AF.Copy,
                    scale=rz2[:],
                )
            nc.sync.dma_start(afeat_d.ap(), afeat[:])

            bag = sb.tile([1, C], F32)
            nc.scalar.activation(bag[:], pzb[0:1, 14:16], AF.Copy, scale=rz2[:])
            nc.vector.tensor_add(bag[:], bag[:], clsb[:])
            nc.sync.dma_start(bag_d.ap(), bag[:])
    nc.compile()
    return nc


_CACHE = {}


def _get_programs():
    if "s1" not in _CACHE:
        _CACHE["s1"] = build_stage1()
        _CACHE["s2"] = build_stage2()
    return _CACHE["s1"], _CACHE["s2"]


def kernel(**inputs):
    x = np.asarray(inputs["x"], np.float32)
    perm = np.asarray(inputs["perm"]).astype(np.int64)
    w_dim = np.asarray(inputs["w_dim"], np.float32)
    att_V_w = np.asarray(inputs["att_V_w"], np.float32)
    att_V_b = np.asarray(inputs["att_V_b"], np.float32)
    att_U_w = np.asarray(inputs["att_U_w"], np.float32)
    att_U_b = np.asarray(inputs["att_U_b"], np.float32)
    att_w = np.asarray(inputs["att_w"], np.float32)
    att_w_b = np.asarray(inputs["att_w_b"], np.float32)
    sub_w = np.asarray(inputs["sub_w"], np.float32)
    sub_b = np.asarray(inputs["sub_b"], np.float32)
    att2_V_w = np.asarray(inputs["att2_V_w"], np.float32)
    att2_V_b = np.asarray(inputs["att2_V_b"], np.float32)
    att2_U_w = np.asarray(inputs["att2_U_w"], np.float32)
    att2_U_b = np.asarray(inputs["att2_U_b"], np.float32)
    att2_w = np.asarray(inputs["att2_w"], np.float32)
    att2_w_b = np.asarray(inputs["att2_w_b"], np.float32)
    cls_w = np.asarray(inputs["cls_w"], np.float32)
    cls_b = np.asarray(inputs["cls_b"], np.float32)

    s1, s2 = _get_programs()

    xp = x[perm]
    dw = (sub_w[:, 1] - sub_w[:, 0])[:, None]
    attw = np.ascontiguousarray(
        np.concatenate([att_V_w, att_U_w, dw, sub_w], axis=1), np.float32
    )
    attw3 = np.ascontiguousarray(np.repeat(att_w, 3, axis=1), np.float32)
    wb3 = np.full((3, 1), float(att_w_b[0]), np.float32)
    vb = np.ascontiguousarray(att_V_b[:, None])
    ub = np.ascontiguousarray(att_U_b[:, None])
    subb3 = np.array([[0.0], [sub_b[0]], [sub_b[1]]], np.float32)
    ident = np.eye(128, dtype=np.float32)

    in_maps = []
    for g in range(G):
        in_maps.append(
            {
                "xg": np.ascontiguousarray(xp[g * NG : (g + 1) * NG]),
                "w_dim": w_dim,
                "attw": attw,
                "attw3": attw3,
                "wb3": wb3,
                "vb": vb,
                "ub": ub,
                "subb3": subb3,
                "ident": ident,
            }
        )
    res1 = run_bass_kernel_spmd(s1, in_maps, core_ids=list(range(G)))
    sub_preds = np.stack([res1.results[g]["pred"][1:3, 0] for g in range(G)])
    pseudo_all = np.concatenate(
        [res1.results[g]["pseudo"] for g in range(G)], axis=0
    )

    a2w = np.ascontiguousarray(
        np.concatenate([att2_V_w, att2_U_w, cls_w], axis=1), np.float32
    )
    b2row = np.ascontiguousarray(
        np.concatenate([att2_V_b, att2_U_b])[None, :], np.float32
    )
    in2 = {
        "pseudo_all": np.ascontiguousarray(pseudo_all),
        "a2w": a2w,
        "a2wrow": np.ascontiguousarray(att2_w[:, 0][None, :]),
        "b2row": b2row,
        "b2s": np.full((128, 1), float(att2_w_b[0]), np.float32),
        "cls_b": np.ascontiguousarray(cls_b[None, :]),
        "zb_ones": np.ones((128, 16), np.float32),
        "ident": ident,
    }
    res2 = run_bass_kernel_spmd(s2, [in2], core_ids=[0])
    bag_pred = res2.results[0]["bag"][0]
    afeat = res2.results[0]["afeat"][0]
    return bag_pred, afeat, sub_preds
